# revision 33
# baseline (speedup 1.0000x reference)
"""Trainium2 Bass kernel for nn_MultiHeadAttention (B=2, S=2048, D=1024, H=16).

Sharding: batch*heads across 8 cores -> each core handles one batch element's
4 heads (core c: b = c//4, heads h0 = (c%4)*4 .. h0+4).

Key idea: the padding mask kills ~half the keys; the host gathers each head's
unmasked key positions (padded to KT tiles of 128) so scores/exp/ctx run over
~9 instead of 16 key tiles.

Structure (fp16 matmuls, f32 PSUM); measured ~165us vs 216us baseline:
  - DMA: few large sprayed transfers at HBM roofline, ordered by first use
    (consts, Wq, xT in 4 query chunks, xg0, Wkv, xg1-3, Wo). The
    projection phase is DMA-bound, so input bytes are the floor there.
  - PE p-state warmup: dummy matmuls on Qt1's zeroed half while xT streams,
    so Q projection starts at the full 2.4GHz clock (TRN2 ramps
    0.65->1.2->2.4GHz and needs ~3us of continuous work).
  - Q projection into two zero-padded transposed tiles (memset, not DMA).
  - FUSED K+V projection: the stationary packs each head's 64 K-dims and
    64 V-dims (halves swapped for odd heads so all drains stay
    partition-aligned), so xg streams through the PE once, not twice.
    K lands in transposed layout (pair-packed Kt), V as VT [vdim, keys],
    PE-transposed into v4 [key, vdim]; T(hl) is emitted under KV(hl+1)'s
    matmuls so drain latency never idles the PE (idle PE drops the clock
    to 1.2GHz for 3us).
  - Attention: one flat 36-step software pipeline across the 4 (pair, half)
    blocks with score matmuls ordered s0-qc0, s0-qc1, s1-qc0, s1-qc1, so
    each score tile refills the moment its exp frees the PSUM slot.
    Steady state hits the ScalarE bound: one 1024-col exp per ~1.0us.
    PSUM is exactly full (score ring 4 banks + ctx ring 4 banks) -- do NOT
    allocate any other PSUM tile between the first and last exp, it breaks
    the ring pairing and stalls the pipeline (~measured +50us).
  - Normalization: softmax sums ride row 64 of the ctx accumulators (ones
    column in v4); 1/sum via DMA-transposed [128, 8] reciprocal (DVE time
    scales with FREE size; any [*, 1024] reciprocal takes ~7us), broadcast
    across partitions on gpsimd (partition_broadcast needs a partition-0
    source), multiply on DVE.  The last block broadcasts via a PE
    ones-matmul instead and runs its hp1 steps first, shortening the only
    non-overlapped drain chain.
  - Output projection in the tail: qt 0-7 (per-half ctxT tiles dodge false
    coarse-dep serialization) run while the last block's chain completes.
Host sums the 4 partial outputs per batch element and adds b_out.
"""

import math
import os

import numpy as np

# Tile's fine-grained (subtile) dependency tracker misses some of this
# kernel's partition-sliced producer->consumer edges (verified empirically:
# per-core divergent results with it on, bit-identical and correct with it
# off). Coarse tile-level deps cost little here and are always safe.
os.environ.setdefault("BY_DEFAULT_DISABLE_SUBTILE_DEPS", "1")

N_HEADS = 16
DIM = 1024
DIM_PER_HEAD = 64
B = 2
S = 2048
SCALE = math.sqrt(DIM_PER_HEAD)
N_CORES = 8
HEADS_PER_CORE = 4

_cache = {}


def _build_program(KT):
    import concourse.tile as tile
    from concourse import bacc, mybir

    f32 = mybir.dt.float32
    fp16 = mybir.dt.float16
    Exp = mybir.ActivationFunctionType.Exp
    SK = KT * 128  # gathered (padded) key count per head

    nc = bacc.Bacc("TRN2", target_bir_lowering=False, debug=False,
                   num_devices=N_CORES)

    xT = nc.dram_tensor("xT", [DIM, S], fp16, kind="ExternalInput").ap()
    xg = nc.dram_tensor("xg", [4, DIM, SK], fp16, kind="ExternalInput").ap()
    Wq = nc.dram_tensor("Wq", [DIM, 256], fp16, kind="ExternalInput").ap()
    Wkv = nc.dram_tensor("Wkv", [DIM, 512], fp16, kind="ExternalInput").ap()
    Wo = nc.dram_tensor("Wo", [256, DIM], fp16, kind="ExternalInput").ap()
    bqk = nc.dram_tensor("bqk", [128, 4], f32, kind="ExternalInput").ap()
    bvT = nc.dram_tensor("bvT", [128, 4], f32, kind="ExternalInput").ap()
    id2 = nc.dram_tensor("id2", [128, 64], fp16, kind="ExternalInput").ap()
    maskT = nc.dram_tensor("maskT", [128, 4 * KT], f32,
                           kind="ExternalInput").ap()
    out_d = nc.dram_tensor("out", [S, DIM], fp16, kind="ExternalOutput").ap()

    with tile.TileContext(nc) as tc:
        with tc.tile_pool(name="const", bufs=1) as cpool, \
             tc.tile_pool(name="wpool", bufs=1) as wpool, \
             tc.tile_pool(name="xgp", bufs=1) as xgp, \
             tc.tile_pool(name="qkv", bufs=1) as qkvp, \
             tc.tile_pool(name="ps", bufs=2, space="PSUM") as ps:

            # ---- input DMAs, ordered by first use; each is one big sprayed
            # transfer so the queues run at HBM roofline ----
            maskT_sb = cpool.tile([128, 4 * KT], f32)
            nc.sync.dma_start(maskT_sb[:], maskT[:])
            bqk_sb = cpool.tile([128, 4], f32)
            nc.sync.dma_start(bqk_sb[:], bqk[:])
            bvT_sb = cpool.tile([128, 4], f32)
            nc.sync.dma_start(bvT_sb[:], bvT[:])
            id_sb = cpool.tile([128, 64], fp16)
            nc.sync.dma_start(id_sb[:], id2[:])
            Wq_sb = wpool.tile([128, 8, 256], fp16)
            nc.sync.dma_start(Wq_sb[:], Wq.rearrange("(c p) j -> p c j", p=128))

            ones_row = cpool.tile([1, 128], fp16)
            nc.gpsimd.memset(ones_row[:], 1.0)

            # Q/K/V targets. Qt1's zero half is memset FIRST on gpsimd: the
            # PE warmup reads it, so it must be ready as early as possible.
            Qt0 = qkvp.tile([128, 2, S], fp16)
            Qt1 = qkvp.tile([128, 2, S], fp16)
            nc.gpsimd.memset(Qt1[0:64, :, :], 0.0)
            nc.gpsimd.memset(Qt0[64:128, :, :], 0.0)
            Kt_p = [qkvp.tile([128, SK], fp16, name=f"Kt_{p}")
                    for p in range(2)]
            v4_h = [qkvp.tile([128, KT, 65], fp16, name=f"v4_{hl}")
                    for hl in range(4)]
            for hl in range(4):
                nc.gpsimd.memset(v4_h[hl][:, :, 64], 1.0)
            ctxT_h = [qkvp.tile([128, 2, 1024], fp16, name=f"ctxT_{half}")
                      for half in range(2)]

            with tc.tile_pool(name="xsub", bufs=1) as xsub:
                # xT in 4 query-chunk tiles so Q proj starts after ~1MB;
                # xg interleaved so each head's gather lands just in time.
                xT_r = xT.rearrange("(c p) s -> p c s", p=128)
                xts = []
                for sc in range(4):
                    t = xsub.tile([128, 8, 512], fp16, name=f"xts_{sc}")
                    nc.sync.dma_start(t[:],
                                      xT_r[:, :, sc * 512:(sc + 1) * 512])
                    xts.append(t)
                xg0 = xgp.tile([128, 8, SK], fp16, name="xg_0")
                nc.sync.dma_start(
                    xg0[:], xg[0].rearrange("(c p) k -> p c k", p=128))
                Wkv_sb = wpool.tile([128, 8, 512], fp16)
                nc.sync.dma_start(
                    Wkv_sb[:], Wkv.rearrange("(c p) j -> p c j", p=128))
                xg_t = [xg0]
                for hl in range(1, 4):
                    t = xgp.tile([128, 8, SK], fp16, name=f"xg_{hl}")
                    nc.sync.dma_start(
                        t[:], xg[hl].rearrange("(c p) k -> p c k", p=128))
                    xg_t.append(t)
                Wo_sb = wpool.tile([128, 2, 1024], fp16)
                nc.sync.dma_start(Wo_sb[:],
                                  Wo.rearrange("(c p) e -> p c e", p=128))

                # ---- PE p-state warmup: TRN2's tensor engine needs ~3us of
                # continuous work to reach 2.4GHz. Grind dummy matmuls on
                # Qt1's zeroed half (ready ~7us, long before any DMA input)
                # so Q projection starts at full clock the moment xT lands.
                for w in range(20):
                    pw = ps.tile([64, 512], f32, tag="a", name=f"pw_{w}")
                    nc.tensor.matmul(
                        pw[:], lhsT=id_sb[0:64, :],
                        rhs=Qt1[0:64, w % 2, (w % 4) * 512:(w % 4 + 1) * 512],
                        start=True, stop=True)

                # ---- Q projection (transposed, zero-padded per head) ----
                for sc in range(4):
                    for p in range(2):
                        ps_t = ps.tile([128, 512], f32,
                                       tag="a" if p == 0 else "ctx",
                                       name=f"pq_{sc}_{p}")
                        for dc in range(8):
                            nc.tensor.matmul(
                                ps_t[:],
                                lhsT=Wq_sb[:, dc, p * 128:(p + 1) * 128],
                                rhs=xts[sc][:, dc, :],
                                start=(dc == 0), stop=(dc == 7))
                        ssl = slice(sc * 512, (sc + 1) * 512)
                        bias = bqk_sb[:, p: p + 1]
                        nc.vector.tensor_scalar_add(
                            Qt0[0:64, p, ssl], ps_t[0:64, :], bias[0:64, :])
                        nc.vector.tensor_scalar_add(
                            Qt1[64:128, p, ssl], ps_t[64:128, :],
                            bias[64:128, :])

            with tc.tile_pool(name="vtp", bufs=2) as vtp, \
                 tc.tile_pool(name="expp", bufs=5) as expp, \
                 tc.tile_pool(name="ctxu", bufs=2) as ctxu, \
                 tc.tile_pool(name="bcp", bufs=4) as bcp, \
                 tc.tile_pool(name="outsb", bufs=4) as outsb, \
                 tc.tile_pool(name="rscr", bufs=2) as rscr:

                # ---- fused K+V projection: stationary packs the head's
                # 64 K-dims and 64 V-dims (halves swapped for odd heads so
                # every drain is partition-aligned), streaming xg ONCE.
                # Out rows: K half -> Kt (transposed-K layout), V half -> VT
                # [vdim, keys], later PE-transposed into v4 [key, vdim]. ----
                VT_of = {}

                def emit_kv(hl, chunks, tags):
                    p, hp = hl // 2, hl % 2
                    kr = slice(hp * 64, hp * 64 + 64)       # K out rows
                    vr = slice(64 - hp * 64, 128 - hp * 64)  # V out rows
                    if hl not in VT_of:
                        VT_of[hl] = vtp.tile([128, SK], fp16, tag="vt",
                                             name=f"vt_{hl}")
                    VT = VT_of[hl]
                    ts = [ps.tile([128, nn], f32, tag=tg,
                                  name=f"pkv_{hl}_{c0}")
                          for (c0, nn), tg in zip(chunks, tags)]
                    for dc in range(8):
                        for t_, (c0, nn) in zip(ts, chunks):
                            nc.tensor.matmul(
                                t_[:],
                                lhsT=Wkv_sb[:, dc, hl * 128:(hl + 1) * 128],
                                rhs=xg_t[hl][:, dc, c0:c0 + nn],
                                start=(dc == 0), stop=(dc == 7))
                    bias = bqk_sb[:, 2 + p: 3 + p]
                    for t_, (c0, nn) in zip(ts, chunks):
                        nc.vector.tensor_scalar_add(
                            Kt_p[p][kr, c0:c0 + nn], t_[kr, :], bias[kr, :])
                        # V drain on ScalarE (idle during projection) so the
                        # next head's matmuls get their PSUM slot back
                        # without queueing behind DVE
                        nc.scalar.add(VT[vr, c0:c0 + nn], t_[vr, :],
                                      bvT_sb[vr, hl:hl + 1])

                def emit_t(hl, groups, tags):
                    # transpose VT 128-key chunks into v4 [key, vdim],
                    # batched so one DVE drain covers several kt
                    hp = hl % 2
                    vr = slice(64 - hp * 64, 128 - hp * 64)
                    VT = VT_of[hl]
                    pts = [ps.tile([128, kl, 64], fp16, tag=tg,
                                   name=f"pt_{hl}_{k0}")
                           for (k0, kl), tg in zip(groups, tags)]
                    for pt_g, (k0, kl) in zip(pts, groups):
                        for j in range(kl):
                            nc.tensor.transpose(
                                pt_g[:, j, :],
                                VT[vr, (k0 + j) * 128:(k0 + j + 1) * 128],
                                id_sb[vr, :])
                    for pt_g, (k0, kl) in zip(pts, groups):
                        nc.vector.tensor_copy(
                            v4_h[hl][:, k0:k0 + kl, 0:64], pt_g[:])

                kh = KT // 2 + 1  # transpose group split (5/4 for KT=9)
                CH512 = []
                c0 = 0
                while c0 < SK:
                    CH512.append((c0, min(512, SK - c0)))
                    c0 += 512
                TGR = [(k0, kl) for k0, kl in ((0, kh), (kh, KT - kh))
                       if kl > 0]
                # All four heads projected serially before attention (PSUM is
                # fully booked during attention, so projection work cannot be
                # injected into the attention steps without stalling the exp
                # ping-pong). T(hl) is emitted under K(hl+1) so the PE grinds
                # matmuls while VT's DVE bias-drains land.
                for hl in range(4):
                    emit_kv(hl, CH512, ["a", "ctx"] * len(CH512))
                    if hl > 0:
                        emit_t(hl - 1, TGR, ["a", "ctx"] * len(TGR))

                # ---- attention: flat 36-step pipeline over 4 blocks ----
                # block b: p = b//2, half = b%2 (pair-major: pair 0 is ready
                # first). Step t: block(t) = t//KT, kt(t) = t%KT.
                NSTEP = 4 * KT

                def blk(t):
                    return (t // KT) // 2, (t // KT) % 2, t % KT

                def emit_scores(t):
                    p, half, kt = blk(t)
                    s0 = ps.tile([128, 1024], f32, tag="a",
                                 name=f"s0_{t}")
                    s1 = ps.tile([128, 1024], f32, tag="a",
                                 name=f"s1_{t}")
                    lhsT = Kt_p[p][:, kt * 128:(kt + 1) * 128]
                    for s_t, qsrc in ((s0, Qt0), (s1, Qt1)):
                        for qc in range(2):
                            q0 = half * 1024 + qc * 512
                            nc.tensor.matmul(
                                s_t[:, qc * 512:(qc + 1) * 512],
                                lhsT=lhsT,
                                rhs=qsrc[:, p, q0:q0 + 512],
                                start=True, stop=True)
                    return s0, s1

                def emit_exp(t, sc_t):
                    p, half, kt = blk(t)
                    ets = []
                    for hp in range(2):
                        et = expp.tile([128, 1024], fp16, tag="et",
                                       name=f"et_{t}_{hp}")
                        nc.scalar.activation(
                            et[:], sc_t[hp][:], Exp,
                            bias=maskT_sb[:, kt * 4 + 2 * p + hp:
                                          kt * 4 + 2 * p + hp + 1],
                            scale=1.0)
                        ets.append(et)
                    return ets

                ctxs_of_block = {}

                def emit_ctx(t, ets):
                    p, half, kt = blk(t)
                    b = t // KT
                    if kt == 0:
                        ctxs_of_block[b] = (
                            ps.tile([65, 1024], f32, tag="ctx", name=f"c0_{b}"),
                            ps.tile([65, 1024], f32, tag="ctx", name=f"c1_{b}"))
                    ctxs = ctxs_of_block[b]
                    for hp in range(2):
                        for qc in range(2):
                            nc.tensor.matmul(
                                ctxs[hp][:, qc * 512:(qc + 1) * 512],
                                lhsT=v4_h[2 * p + hp][:, kt, :],
                                rhs=ets[hp][:, qc * 512:(qc + 1) * 512],
                                start=(kt == 0), stop=(kt == KT - 1))

                norm_q = []

                def emit_drain(b, last=False):
                    # block b finished accumulating: move ctx out of PSUM,
                    # compute 1/rowsum (on a DMA-transposed [128, 8] view --
                    # DVE op time scales with FREE size, so any [*, 1024]
                    # reciprocal would take ~7us), broadcast it across
                    # partitions, queue the normalize multiplies. For the
                    # last block the broadcast uses a PE ones-matmul instead
                    # of gpsimd (shorter latency, and the score ring is free
                    # in the tail).
                    p, half = b // 2, b % 2
                    ctxs = ctxs_of_block.pop(b)
                    ctxUs, rss, bcs = [], [], []
                    for hp in range(2):
                        ctxU = ctxu.tile([65, 1024], fp16, tag="cu", bufs=4,
                                         name=f"cu_{b}_{hp}")
                        nc.vector.tensor_copy(ctxU[:], ctxs[hp][:])
                        s128 = rscr.tile([128, 8], fp16, tag="sm",
                                         name=f"sm_{b}_{hp}")
                        nc.sync.dma_start(s128[:], ctxU[64:65, :])
                        r128 = rscr.tile([128, 8], fp16, tag="rc",
                                         name=f"rc_{b}_{hp}")
                        with nc.allow_low_precision(
                                reason="fp16 softmax-sum reciprocal"):
                            nc.vector.reciprocal(r128[:], s128[:])
                        rs_t = rscr.tile([1, 1024], fp16, tag="rs",
                                         name=f"rs_{b}_{hp}")
                        nc.sync.dma_start(rs_t[:], r128[:])
                        ctxUs.append(ctxU)
                        rss.append(rs_t)
                        if not last:
                            bc_t = bcp.tile([64, 1024], fp16, tag="bc",
                                            bufs=4, name=f"bc_{b}_{hp}")
                            nc.gpsimd.partition_broadcast(bc_t[:],
                                                          rs_t[0:1, :])
                            bcs.append(bc_t)

                    box = {}

                    def step(j):
                        hp_, qc = j // 2, j % 2
                        if hp_ == 0:
                            tgt = ctxT_h[half][0:64, p,
                                              qc * 512:(qc + 1) * 512]
                        else:
                            if "t" not in box:
                                box["t"] = ctxu.tile([64, 1024], fp16,
                                                     tag="cn", bufs=2,
                                                     name=f"cn_{b}")
                            tgt = box["t"][0:64, qc * 512:(qc + 1) * 512]
                        if last:
                            pb_t = ps.tile([128, 512], f32, tag="a",
                                           name=f"pb_{b}_{j}")
                            nc.tensor.matmul(
                                pb_t[:], lhsT=ones_row[0:1, :],
                                rhs=rss[hp_][0:1, qc * 512:(qc + 1) * 512],
                                start=True, stop=True)
                            mul_rhs = pb_t[0:64, :]
                        else:
                            mul_rhs = bcs[hp_][0:64,
                                               qc * 512:(qc + 1) * 512]
                        nc.vector.tensor_mul(
                            tgt, ctxUs[hp_][0:64, qc * 512:(qc + 1) * 512],
                            mul_rhs)
                        if hp_ == 1 and qc == 1:
                            nc.sync.dma_start(
                                ctxT_h[half][64:128, p, :], box["t"][0:64, :])

                    order = (2, 3, 0, 1) if last else (0, 1, 2, 3)
                    norm_q.extend([lambda j=j: step(j) for j in order])

                sc_cur = emit_scores(0)
                emit_t(3, TGR, ["a", "ctx"] * len(TGR))
                prev = None  # (t-1, ets)
                for t in range(NSTEP):
                    ets = emit_exp(t, sc_cur)
                    if prev is not None:
                        emit_ctx(prev[0], prev[1])
                        if prev[0] % KT == KT - 1:
                            emit_drain(prev[0] // KT)
                    if t < NSTEP - 1:
                        sc_cur = emit_scores(t + 1)
                    prev = (t, ets)
                    if norm_q and 2 <= (t % KT) <= 5:
                        norm_q.pop(0)()
                emit_ctx(prev[0], prev[1])
                emit_drain(3, last=True)

                # ---- output projection tail ----
                # qt 0-7 read ctxT half 0 (finished during block 3), so they
                # run while block 3's drain chain + norms (feeding half 1)
                # complete; the norm pb matmuls then find their rs operands
                # already landed and don't stall the PE queue.
                def emit_po(qt):
                    half, c = qt // 8, qt % 8
                    tag = "a" if qt % 2 == 0 else "ctx"
                    po = ps.tile([128, 1024], f32, tag=tag, name=f"po_{qt}")
                    for p_ in range(2):
                        for ec in range(2):
                            nc.tensor.matmul(
                                po[:, ec * 512:(ec + 1) * 512],
                                lhsT=ctxT_h[half][:, p_,
                                                  c * 128:(c + 1) * 128],
                                rhs=Wo_sb[:, p_, ec * 512:(ec + 1) * 512],
                                start=(p_ == 0), stop=(p_ == 1))
                    ob = outsb.tile([128, 1024], fp16, tag="ob",
                                    name=f"ob_{qt}")
                    if qt % 2 == 0:
                        nc.vector.tensor_copy(ob[:], po[:])
                    else:
                        nc.scalar.copy(ob[:], po[:])
                    nc.sync.dma_start(out_d[qt * 128:(qt + 1) * 128, :],
                                      ob[:])

                for qt in range(8):
                    emit_po(qt)
                for st_fn in norm_q:
                    st_fn()
                for qt in range(8, 16):
                    emit_po(qt)

    nc.compile()
    return nc


def get_program(KT=9):
    key = ("nc", KT)
    if key not in _cache:
        _cache[key] = _build_program(KT)
    return _cache[key]


def make_in_maps(query, mask, W_qkv, b_qkv, W_out, b_out):
    query = np.asarray(query, dtype=np.float32)
    mask = np.asarray(mask)
    W_qkv = np.asarray(W_qkv, dtype=np.float32)
    b_qkv = np.asarray(b_qkv, dtype=np.float32)
    W_out = np.asarray(W_out, dtype=np.float32)
    bf = np.float16

    W3 = W_qkv.reshape(DIM, N_HEADS, DIM_PER_HEAD, 3)
    b3 = b_qkv.reshape(N_HEADS, DIM_PER_HEAD, 3)
    m2 = np.asarray(mask)[:, 0, :]  # [32, 2048] True = masked
    KT = max(1, int(np.ceil((~m2).sum(axis=1).max() / 128)))
    SK = KT * 128

    in_maps = []
    for c in range(N_CORES):
        b = c // 4
        h0 = (c % 4) * HEADS_PER_CORE
        hs = slice(h0, h0 + HEADS_PER_CORE)
        Wq_c = np.ascontiguousarray(
            W3[:, hs, :, 0].reshape(DIM, 256) / SCALE).astype(bf)
        Wk_c = W3[:, hs, :, 1].reshape(DIM, 4, 64)
        Wv_c = W3[:, hs, :, 2].reshape(DIM, 4, 64)
        # fused K|V stationary: per head 128 cols, K half and V half swapped
        # for odd heads so every PSUM drain is partition-aligned
        Wkv_c = np.zeros((DIM, 4, 2, 64), dtype=np.float32)
        for hl in range(4):
            Wkv_c[:, hl, hl % 2, :] = Wk_c[:, hl, :]
            Wkv_c[:, hl, 1 - hl % 2, :] = Wv_c[:, hl, :]
        Wkv_c = np.ascontiguousarray(Wkv_c.reshape(DIM, 512)).astype(bf)
        bq_c = (b3[hs, :, 0].reshape(256) / SCALE).astype(np.float32)
        bk_c = b3[hs, :, 1].reshape(256).astype(np.float32)
        bvT_c = np.ascontiguousarray(
            np.tile(b3[hs, :, 2].T, (2, 1))).astype(np.float32)  # [128, 4]
        id2_c = np.ascontiguousarray(
            np.concatenate([np.eye(64), np.eye(64)], axis=0)).astype(bf)
        bqk_c = np.ascontiguousarray(
            np.stack([bq_c[:128], bq_c[128:], bk_c[:128], bk_c[128:]], axis=1))
        Wo_c = np.ascontiguousarray(
            W_out[h0 * 64:(h0 + 4) * 64, :]).astype(bf)
        xT_c = np.ascontiguousarray(query[b].T).astype(bf)

        xg_c = np.zeros((4, DIM, SK), dtype=bf)
        maskT_c = np.zeros((128, 4 * KT), dtype=np.float32)
        for hl in range(4):
            bh = b * N_HEADS + h0 + hl
            idx = np.nonzero(~m2[bh])[0]
            n = len(idx)
            idx_pad = np.zeros(SK, dtype=np.int64)
            idx_pad[:n] = idx
            xg_c[hl] = xT_c[:, idx_pad]
            padded = np.arange(SK) >= n  # [SK] True = padding slot
            maskT_c[:, hl::4] = np.where(
                padded.reshape(KT, 128).T, np.float32(-30000.0),
                np.float32(0.0))
        in_maps.append({
            "xT": xT_c, "xg": xg_c, "Wq": Wq_c, "Wkv": Wkv_c,
            "Wo": Wo_c, "bqk": bqk_c, "bvT": bvT_c, "id2": id2_c,
            "maskT": maskT_c,
        })
    return in_maps, KT


def gather_outputs(results, b_out):
    b_out = np.asarray(b_out, dtype=np.float32)
    out = np.zeros((B, S, DIM), dtype=np.float32)
    for c in range(N_CORES):
        out[c // 4] += results[c]["out"].astype(np.float32)
    out += b_out[None, None, :]
    return out


def kernel(query, mask, W_qkv, b_qkv, W_out, b_out):
    from concourse.bass_utils import run_bass_kernel_spmd

    in_maps, KT = make_in_maps(query, mask, W_qkv, b_qkv, W_out, b_out)
    nc = get_program(KT)
    res = run_bass_kernel_spmd(nc, in_maps, list(range(N_CORES)))
    return gather_outputs(res.results, b_out)
